# revision 1
# baseline (speedup 1.0000x reference)
"""Trainium2 Bass kernel for nn_LocalFeatureExtractor (gnn_message_passing).

Math: with per-node features x[b,n,:] (C=128) and K=10 gathered neighbors,
    out = x @ W1^T + W1_b + (conv(feats) + Wc_b) @ W2^T + W2_b
collapses algebraically (fold the two dense layers around the conv) to
    out[b,n] = x[b,n] @ A + sum_k x[b, adj[b,n,k]] @ M_k + bias
with A = W1^T + (W2 Wc_0)^T, M_k = (W2 Wc_k)^T, bias = W1_b + W2_b + W2 Wc_b.

Sharding: data-parallel over batch B=8 -> one graph per NeuronCore.
Device layout: x transposed to [C=128 partitions, N] so the per-edge gather
is a free-dim column gather done on GPSIMD (ap_gather) straight out of
SBUF-resident x^T, feeding the tensor engine in matmul-rhs orientation.
"""

import numpy as np

import concourse.bass as bass
import concourse.mybir as mybir
from concourse import bacc
from concourse.tile import TileContext
from concourse.bass_utils import run_bass_kernel_spmd

B, N, C, K = 8, 20000, 128, 10
N_CORES = 8
TILE = 500                  # nodes per PSUM tile (<=512 fp32 fr ee dim)
NT = N // TILE              # 40
KPC = 2                     # neighbor slots per gather chunk
CH = K // KPC               # 5 gather chunks per tile (ap_gather is ~1.5x
JC = 1024                   # faster per idx at ~1K idxs than at ~5K)
SLOTC = JC // 16            # 64 -> per-chunk idx slices stay 4B-aligned

_dt = mybir.dt


def build(n_cores=N_CORES, reps=1, mm_dtype=_dt.float32, n=N, tile=TILE,
          gbufs=2, psbufs=2, obufs=3, nt_limit=None):
    """Build + compile the per-core Bass program (SPMD: same program, 8 cores)."""
    nt = n // tile
    if nt_limit is not None:
        nt = min(nt, nt_limit)
    jc = -(-(KPC * tile) // 32) * 32
    slot = jc // 16

    nc = bacc.Bacc("TRN2", target_bir_lowering=False, debug=False,
                   num_devices=n_cores)
    # inputs feeding matmuls are declared float32r end-to-end: the walrus BIR
    # verifier requires fp32r matmul operands to be produced as fp32r
    xT = nc.dram_tensor("xT", [C, n], mm_dtype, kind="ExternalInput").ap()
    idx = nc.dram_tensor("idx", [C, nt * CH * slot], _dt.int16, kind="ExternalInput").ap()
    wts = nc.dram_tensor("wts", [C, (K + 1) * C], mm_dtype, kind="ExternalInput").ap()
    bias = nc.dram_tensor("bias", [C, 1], _dt.float32, kind="ExternalInput").ap()
    outT = nc.dram_tensor("outT", [C, n], _dt.float32, kind="ExternalOutput").ap()

    with TileContext(nc) as tc:
        with tc.tile_pool(name="const", bufs=1) as cpool, \
             tc.tile_pool(name="gath", bufs=gbufs) as gpool, \
             tc.tile_pool(name="psum", bufs=psbufs, space="PSUM") as ppool, \
             tc.tile_pool(name="outp", bufs=obufs) as opool:
            xT_t = cpool.tile([C, n], mm_dtype)
            idx_t = cpool.tile([C, nt * CH * slot], _dt.int16)
            wts_t = cpool.tile([C, (K + 1) * C], mm_dtype)
            bias_t = cpool.tile([C, 1], _dt.float32)
            nc.sync.dma_start(out=xT_t[:], in_=xT[:])
            nc.sync.dma_start(out=idx_t[:], in_=idx[:])
            nc.sync.dma_start(out=wts_t[:], in_=wts[:])
            nc.sync.dma_start(out=bias_t[:], in_=bias[:])

            for _rep in range(reps):
                for t in range(nt):
                    gs = []
                    for c in range(CH):
                        g = gpool.tile([C, jc], mm_dtype, tag="g%d" % c)
                        nc.gpsimd.ap_gather(
                            out_ap=g[:],
                            in_ap=xT_t[:],
                            idxs_ap=idx_t[:, (t * CH + c) * slot:
                                           (t * CH + c + 1) * slot],
                            channels=C, num_elems=n, d=1, num_idxs=jc,
                        )
                        gs.append(g)
                    ps = ppool.tile([C, tile], _dt.float32)
                    nc.tensor.matmul(
                        out=ps[:],
                        lhsT=wts_t[:, 0:C],
                        rhs=xT_t[:, t * tile:(t + 1) * tile],
                        start=True, stop=False,
                    )
                    for k in range(1, K + 1):
                        c, o = (k - 1) // KPC, ((k - 1) % KPC) * tile
                        nc.tensor.matmul(
                            out=ps[:],
                            lhsT=wts_t[:, k * C:(k + 1) * C],
                            rhs=gs[c][:, o:o + tile],
                            start=False, stop=(k == K),
                        )
                    o = opool.tile([C, tile], _dt.float32)
                    nc.scalar.activation(
                        o[:], ps[:], mybir.ActivationFunctionType.Identity,
                        bias=bias_t[:], scale=1.0,
                    )
                    nc.sync.dma_start(out=outT[:, t * tile:(t + 1) * tile], in_=o[:])
    nc.compile()
    return nc


def fold_weights(W1_w, W1_b, Wc_w, Wc_b, W2_w, W2_b):
    """Collapse Linear->Conv1d->Linear into 11 [C,C] mats + one bias."""
    W2 = W2_w.astype(np.float64)
    M = np.einsum('de,eck->cdk', W2, Wc_w.astype(np.float64))
    M[:, :, 0] += W1_w.T.astype(np.float64)
    wts = np.concatenate([M[:, :, k] for k in range(K + 1)], axis=1)
    bias = W1_b.astype(np.float64) + W2_b.astype(np.float64) + W2 @ Wc_b.astype(np.float64)
    return wts.astype(np.float32), bias.astype(np.float32).reshape(C, 1)


def make_idx(adj_b, n=N, tile=TILE):
    """adj[b] [n,K] -> wrapped int16 gather-index layout [128, nt*slot].

    ap_gather semantics: each GPSIMD core (16 partitions) reads its own
    partition block; index j of the gather comes from partition j%16,
    slot j//16.  Same index list needed on all 8 cores -> replicate 8x.
    Column order per tile: j = k*tile + i  ->  g[:, k*tile+i] = xT[:, adj[n0+i, k]].
    """
    nt = n // tile
    jc = -(-(KPC * tile) // 32) * 32
    slot = jc // 16
    a = np.asarray(adj_b).reshape(nt, tile, K).astype(np.int16)
    j = np.transpose(a, (0, 2, 1)).reshape(nt, CH, KPC * tile)  # k-major chunks
    jfull = np.zeros((nt, CH, jc), dtype=np.int16)
    jfull[:, :, :KPC * tile] = j
    out = jfull.reshape(nt, CH, slot, 16).transpose(0, 1, 3, 2)  # [nt,CH,16,slot]
    # assemble [128, nt*CH*slot]: partition p = 16c+q uses row q everywhere
    blk = out.transpose(2, 0, 1, 3).reshape(16, nt * CH * slot)
    return np.tile(blk, (8, 1)).copy()                           # replicate 8x


def prep_core_inputs(x, adj_mat, wts, bias):
    """Per-core (per-graph) input maps for the SPMD launch."""
    maps = []
    for b in range(B):
        maps.append({
            "xT": np.ascontiguousarray(np.asarray(x[b]).T).astype(np.float32, copy=False),
            "idx": make_idx(adj_mat[b]),
            "wts": wts,
            "bias": bias,
        })
    return maps


_NC_CACHE = {}


def kernel(x, adj_mat, W1_w, W1_b, Wc_w, Wc_b, W2_w, W2_b):
    x = np.asarray(x)
    adj_mat = np.asarray(adj_mat)
    wts, bias = fold_weights(np.asarray(W1_w), np.asarray(W1_b), np.asarray(Wc_w),
                             np.asarray(Wc_b), np.asarray(W2_w), np.asarray(W2_b))
    if "nc" not in _NC_CACHE:
        _NC_CACHE["nc"] = build()
    nc = _NC_CACHE["nc"]
    in_maps = prep_core_inputs(x, adj_mat, wts, bias)
    res = run_bass_kernel_spmd(nc, in_maps, list(range(N_CORES)))
    out = np.empty((B, N, C), dtype=np.float32)
    for b in range(B):
        out[b] = res.results[b]["outT"].T
    return out



# revision 8
# speedup vs baseline: 6.6674x; 6.6674x over previous
"""Trainium2 Bass kernel for nn_LocalFeatureExtractor (gnn_message_passing).

Math: with per-node features x[b,n,:] (C=128) and K=10 gathered neighbors,
    out = x @ W1^T + W1_b + (conv(feats) + Wc_b) @ W2^T + W2_b
collapses algebraically (fold the two dense layers around the conv) to
    out[b,n] = x[b,n] @ A + sum_k x[b, adj[b,n,k]] @ M_k + bias
with A = W1^T + (W2 Wc_0)^T, M_k = (W2 Wc_k)^T, bias = W1_b + W2_b + W2 Wc_b.

Sharding: data-parallel over batch B=8 -> one graph per NeuronCore.

The 200K per-core column gathers run on the 16 DMA engines via SWDGE
dma_gather (transpose mode): each int16 index pulls one 256B bf16 row of x
straight out of DRAM and transposes it into matmul-rhs orientation
[C partitions, nodes] in SBUF. GPSIMD only generates descriptors
(~0.34ns/idx), so gather throughput is DMA-bus-bound (~1.42ns/column)
instead of GPSIMD-bound (~28us per gather instruction in the ap_gather
formulation). Matmuls are bf16 (tolerance 2e-2), accumulate in fp32 PSUM.
"""

import numpy as np

import concourse.bass as bass
import concourse.mybir as mybir
from concourse import bacc
from concourse.tile import TileContext
from concourse.bass_utils import run_bass_kernel_spmd

B, N, C, K = 8, 20000, 128, 10
N_CORES = 8

TGD = 500                    # nodes per dma_gather group (= psum chunk)
GD = N // TGD                # 40 groups
NDV = K * TGD                # 5000 valid idxs per dma_gather
NID = -(-NDV // 128) * 128   # padded to 5120 (transpose needs %128==0)
CHUNK = 500                  # psum tile columns (<=512 fp32)

_dt = mybir.dt


def build(n_cores=N_CORES, reps=1, n=N, tgd=TGD, bbufs=3, psbufs=4, obufs=4):
    """Build + compile the per-core Bass program (SPMD: same program, 8 cores)."""
    bf16 = _dt.bfloat16
    gd = n // tgd
    ndv = K * tgd
    nid = -(-ndv // 128) * 128
    chunk = tgd

    nc = bacc.Bacc("TRN2", target_bir_lowering=False, debug=False,
                   num_devices=n_cores)
    xT = nc.dram_tensor("xT", [C, n], bf16, kind="ExternalInput").ap()
    xr = nc.dram_tensor("xr", [n, C], bf16, kind="ExternalInput").ap()
    idxd = nc.dram_tensor("idxd", [C, gd * (nid // 16)], _dt.int16,
                          kind="ExternalInput").ap()
    wts = nc.dram_tensor("wts", [C, (K + 1) * C], bf16, kind="ExternalInput").ap()
    bias = nc.dram_tensor("bias", [C, 1], _dt.float32, kind="ExternalInput").ap()
    outT = nc.dram_tensor("outT", [C, n], bf16, kind="ExternalOutput").ap()

    with TileContext(nc) as tc:
        with tc.tile_pool(name="const", bufs=1) as cpool, \
             tc.tile_pool(name="gd_pool", bufs=bbufs) as bpool, \
             tc.tile_pool(name="psum", bufs=psbufs, space="PSUM") as ppool, \
             tc.tile_pool(name="outp", bufs=obufs) as opool:
            xT_t = cpool.tile([C, n], bf16)
            idxd_t = cpool.tile([C, gd * (nid // 16)], _dt.int16)
            wts_t = cpool.tile([C, (K + 1) * C], bf16)
            bias_t = cpool.tile([C, 1], _dt.float32)
            nc.sync.dma_start(out=idxd_t[:], in_=idxd[:])
            nc.sync.dma_start(out=xT_t[:], in_=xT[:])
            nc.sync.dma_start(out=wts_t[:], in_=wts[:])
            nc.sync.dma_start(out=bias_t[:], in_=bias[:])

            for _rep in range(reps):
                for g in range(gd):
                    bt = bpool.tile([C, 1, nid], bf16, tag="b",
                                    name="b%d_%d" % (_rep, g))
                    nc.gpsimd.dma_gather(
                        out_ap=bt[:],
                        in_ap=xr[:],
                        idxs_ap=idxd_t[:, g * (nid // 16):(g + 1) * (nid // 16)],
                        num_idxs=nid,
                        num_idxs_reg=ndv,
                        elem_size=C,
                        transpose=True,
                        single_packet=False,
                    )
                    ps = ppool.tile([C, chunk], _dt.float32, tag="ps",
                                    name="ps%d_%d" % (_rep, g))
                    nc.tensor.matmul(
                        out=ps[:],
                        lhsT=wts_t[:, 0:C],
                        rhs=xT_t[:, g * chunk:(g + 1) * chunk],
                        start=True, stop=False,
                    )
                    for k in range(1, K + 1):
                        nc.tensor.matmul(
                            out=ps[:],
                            lhsT=wts_t[:, k * C:(k + 1) * C],
                            rhs=bt[:, 0, (k - 1) * tgd:k * tgd],
                            start=False, stop=(k == K),
                        )
                    o = opool.tile([C, chunk], bf16, tag="o",
                                   name="o%d_%d" % (_rep, g))
                    nc.scalar.activation(
                        o[:], ps[:], mybir.ActivationFunctionType.Identity,
                        bias=bias_t[:], scale=1.0,
                    )
                    nc.sync.dma_start(out=outT[:, g * chunk:(g + 1) * chunk],
                                      in_=o[:])
    nc.compile()
    return nc


def fold_weights(W1_w, W1_b, Wc_w, Wc_b, W2_w, W2_b):
    """Collapse Linear->Conv1d->Linear into 11 [C,C] mats + one bias."""
    W2 = W2_w.astype(np.float64)
    M = np.einsum('de,eck->cdk', W2, Wc_w.astype(np.float64))
    M[:, :, 0] += W1_w.T.astype(np.float64)
    wts = np.concatenate([M[:, :, k] for k in range(K + 1)], axis=1)
    bias = W1_b.astype(np.float64) + W2_b.astype(np.float64) + W2 @ Wc_b.astype(np.float64)
    return wts.astype(np.float32), bias.astype(np.float32).reshape(C, 1)


def make_idx_dma(adj_b):
    """dma_gather idx tensor [128, GD*(NID//16)] int16: per 500-node group,
    k-major (j = (k-1)*TGD + i), padded to NID with -1, wrapped so idx j sits
    at (partition j%16, slot j//16), 16-row block replicated to 128 rows."""
    a = np.asarray(adj_b).reshape(GD, TGD, K)
    j = np.transpose(a, (0, 2, 1)).reshape(GD, NDV)    # k-major per group
    jf = np.full((GD, NID), -1, dtype=np.int16)
    jf[:, :NDV] = j
    w = jf.reshape(GD, NID // 16, 16).transpose(0, 2, 1)  # [GD, 16, slots]
    blk = w.transpose(1, 0, 2).reshape(16, GD * (NID // 16))
    return np.tile(blk, (8, 1)).copy()


def prep_core_inputs(x, adj_mat, wts, bias):
    """Per-core (per-graph) input maps for the SPMD launch."""
    bf16 = _dt.np(_dt.bfloat16)
    wts_bf = np.ascontiguousarray(wts.astype(bf16))
    maps = []
    for b in range(B):
        xr = np.ascontiguousarray(np.asarray(x[b]).astype(bf16))
        maps.append({
            "xT": np.ascontiguousarray(xr.T),
            "xr": xr,
            "idxd": make_idx_dma(adj_mat[b]),
            "wts": wts_bf,
            "bias": bias.astype(np.float32),
        })
    return maps


_NC_CACHE = {}


def kernel(x, adj_mat, W1_w, W1_b, Wc_w, Wc_b, W2_w, W2_b):
    x = np.asarray(x)
    adj_mat = np.asarray(adj_mat)
    wts, bias = fold_weights(np.asarray(W1_w), np.asarray(W1_b), np.asarray(Wc_w),
                             np.asarray(Wc_b), np.asarray(W2_w), np.asarray(W2_b))
    if "nc" not in _NC_CACHE:
        _NC_CACHE["nc"] = build()
    nc = _NC_CACHE["nc"]
    in_maps = prep_core_inputs(x, adj_mat, wts, bias)
    res = run_bass_kernel_spmd(nc, in_maps, list(range(N_CORES)))
    out = np.empty((B, N, C), dtype=np.float32)
    for b in range(B):
        out[b] = res.results[b]["outT"].T.astype(np.float32)
    return out
